# revision 33
# baseline (speedup 1.0000x reference)
"""GAT exercise->KC attention layer on 8 TRN2 NeuronCores.

Sharding: exercise rows split 8 ways (2500/core, padded to 2560); kc_h/W1/E/a
replicated. Row-local softmax + aggregation => no collectives.

Layout: everything on-chip runs "transposed" — kc chunks (8 x 128) on
partitions, exercise rows on the free axis. The host ships adj and ex already
transposed, so the kernel needs no per-tile PE transposes at all:

  s1B  [p, r] = W_A1MAT.T @ exT          (every partition = s1[r])
  e_c  [k_c, r] = prelu(s1B + s2[k_c])   (ACT, per-partition bias)
  t_c  = exp(e_c) -> bf16                (ACT)
  tm_c = t_c * adjT_c                    (DVE, bf16)
  rowsumB [p, r] = ONES.T @ tm           (every partition = rowsum[r])
  recipB = exp(-ln(rowsumB))             (ACT; vector.reciprocal is per-lane)
  aggT [fo, r] = sum_c kcWh_c.T @ tm_c   (bf16 matmuls)
  out  = elu(aggT * exEhT * recipB)      ([fo, r]; host transposes back)
"""

import numpy as np
import ml_dtypes
from contextlib import ExitStack

from concourse import bass, bacc, tile, mybir
from concourse.bass_utils import run_bass_kernel_spmd

F32 = mybir.dt.float32
BF16 = mybir.dt.bfloat16

N_EX, N_KC, F = 20000, 1024, 128
N_CORES = 8
ROWS = N_EX // N_CORES          # 2500 rows per core
ROWS_PAD = 2560                 # pad -> uniform tiles
GR = 512                        # rows per group (1 PSUM bank for f32 outputs)
NG = (ROWS_PAD + GR - 1) // GR  # 5 groups
NCH = N_KC // 128               # 8 kc chunks

LEAKY_SLOPE = 0.2
AF = mybir.ActivationFunctionType
ALU = mybir.AluOpType


def build_kernel_body(ctx: ExitStack, tc: "tile.TileContext", ins: dict, outs: dict,
                      sim_safe: bool = False, reps: int = 1):
    nc = tc.nc
    adjT = ins["adjT"]      # [N_KC, ROWS_PAD] bf16 (0/1; pad rows = 1)
    exT = ins["exT"]        # [F, ROWS_PAD] bf16 (pad rows = 0)
    kc = ins["kc"]          # [N_KC, F] f32
    w1 = ins["w1"]          # [F, F] f32
    emb = ins["emb"]        # [F, F] bf16  (E matrix, pre-cast)
    a1 = ins["a1"]          # [F, 1] f32
    a2 = ins["a2"]          # [F, 1] f32
    idf = ins["idf"]        # [128, 128] f32 identity
    onesb = ins["onesb"]    # [128, 128] bf16 all-ones
    out = outs["outT"]      # [F, ROWS_PAD] f32 (host transposes back)

    const = ctx.enter_context(tc.tile_pool(name="const", bufs=1))
    pctx = ctx.enter_context(ExitStack())
    prolog = pctx.enter_context(tc.tile_pool(name="prolog", bufs=2))
    pp = pctx.enter_context(tc.tile_pool(name="pp", bufs=2, space="PSUM"))

    # ---- constants ----
    w1_sb = const.tile([F, F], F32, tag="w1")
    nc.sync.dma_start(w1_sb[:], w1)
    emb_sb = const.tile([F, F], BF16, tag="emb")
    nc.sync.dma_start(emb_sb[:], emb)
    a1_sb = const.tile([F, 1], F32, tag="a1")
    nc.sync.dma_start(a1_sb[:], a1)
    a2_sb = const.tile([F, 1], F32, tag="a2")
    nc.sync.dma_start(a2_sb[:], a2)
    idf_sb = const.tile([128, 128], F32, tag="idf")
    nc.sync.dma_start(idf_sb[:], idf)
    ones_sb = const.tile([128, 128], BF16, tag="onesb")
    nc.sync.dma_start(ones_sb[:], onesb)

    # exercise/adj shards (already transposed on host)
    exT_sb = const.tile([F, ROWS_PAD], BF16, tag="exT")
    nc.sync.dma_start(exT_sb[:], exT)
    adjT_sb = const.tile([128, NCH, ROWS_PAD], BF16, tag="adjT")
    adjT_r = adjT.rearrange("(c p) r -> p c r", p=128)
    for c in range(0, NCH, 2):
        nc.sync.dma_start(adjT_sb[:, c:c + 2, :], adjT_r[:, c:c + 2, :])

    # ---- kc-derived constants ----
    # kc_hT [fi, k] f32 via PE transposes (one DMA for kc)
    kc_all = prolog.tile([128, NCH, F], F32, tag="kc_all")
    nc.sync.dma_start(kc_all[:], kc.rearrange("(c p) f -> p c f", p=128))
    kc_hT = prolog.tile([F, N_KC], F32, tag="kc_hT")
    for c in range(NCH):
        tp = pp.tile([128, 128], F32, tag="ptrans")
        nc.tensor.transpose(tp[:], kc_all[:, c, :], idf_sb[:])
        nc.vector.tensor_copy(kc_hT[:, c * 128:(c + 1) * 128], tp[:])

    # kcWh chunks [k_c, fo] bf16 (aggregation lhsT)
    kcWh_bf = const.tile([128, N_KC], BF16, tag="kcWh")
    for c in range(NCH):
        mp = pp.tile([128, 128], F32, tag="ptrans")
        nc.tensor.matmul(mp[:], lhsT=kc_hT[:, c * 128:(c + 1) * 128], rhs=w1_sb[:],
                         start=True, stop=True)
        nc.scalar.copy(kcWh_bf[:, c * 128:(c + 1) * 128], mp[:])

    # kcWhT [fo, k] f32 -> s2 chunks [k_c, 1] f32 (prelu bias)
    kcWhT = prolog.tile([F, N_KC], F32, tag="kcWhT")
    for h in range(2):
        sl = slice(h * 512, (h + 1) * 512)
        mp = pp.tile([128, 512], F32, tag="pwide")
        nc.tensor.matmul(mp[:], lhsT=w1_sb[:], rhs=kc_hT[:, sl], start=True, stop=True)
        nc.vector.tensor_copy(kcWhT[:, sl], mp[:])
    s2_sb = const.tile([128, NCH], F32, tag="s2")
    for c in range(NCH):
        sp = pp.tile([128, 128], F32, tag="ptrans")
        nc.tensor.matmul(sp[:, :1], lhsT=kcWhT[:, c * 128:(c + 1) * 128],
                         rhs=a2_sb[:], start=True, stop=True)
        nc.vector.tensor_copy(s2_sb[:, c:c + 1], sp[:, :1])

    # w_a1 = W1 @ a1 replicated into a [fi, 128] bf16 stationary: one matmul
    # then broadcasts s1 across partitions for free.
    w1t_p = pp.tile([128, 128], F32, tag="ptrans")
    nc.tensor.transpose(w1t_p[:], w1_sb[:], idf_sb[:])
    w1t_sb = prolog.tile([F, F], F32, tag="w1t")
    nc.vector.tensor_copy(w1t_sb[:], w1t_p[:])
    wa1_p = pp.tile([128, 128], F32, tag="ptrans")
    nc.tensor.matmul(wa1_p[:, :1], lhsT=w1t_sb[:], rhs=a1_sb[:],
                     start=True, stop=True)
    wa1_col = prolog.tile([F, 1], F32, tag="wa1_col")
    nc.vector.tensor_copy(wa1_col[:], wa1_p[:, :1])
    onesf = prolog.tile([128, 128], F32, tag="onesf")
    nc.vector.memset(onesf[:], 1.0)
    wa1mat = const.tile([F, F], BF16, tag="wa1mat")
    nc.scalar.activation(wa1mat[:], onesf[:], AF.Copy, scale=wa1_col[:])

    pctx.close()

    # ---- main-loop pools (SBUF is tight: big [128, 2560] strips tag-share) ----
    elp = ctx.enter_context(tc.tile_pool(name="elp", bufs=3 if sim_safe else 1))
    tp_ = ctx.enter_context(tc.tile_pool(name="tp", bufs=2))
    tmtp = ctx.enter_context(tc.tile_pool(name="tmtp", bufs=1))
    fin = ctx.enter_context(tc.tile_pool(name="fin", bufs=4))
    exep = ctx.enter_context(tc.tile_pool(name="exep", bufs=1))
    rcb = ctx.enter_context(tc.tile_pool(name="rcb", bufs=2))
    ps_s1b = ctx.enter_context(tc.tile_pool(name="ps_s1b", bufs=2, space="PSUM"))
    ps_rsb = ctx.enter_context(tc.tile_pool(name="ps_rsb", bufs=2, space="PSUM"))
    ps_agg = ctx.enter_context(tc.tile_pool(name="ps_agg", bufs=2, space="PSUM"))
    ps_exe = ctx.enter_context(tc.tile_pool(name="ps_exe", bufs=2, space="PSUM"))

    for it in range(reps):
        # s1B [p, r] / exEhT [fo, r]: per-group matmuls (PSUM bank = 512 f32),
        # each copied into a full-width SBUF strip right away.
        s1B_sb = rcb.tile([128, ROWS_PAD], F32, tag="strip")
        exe_sb = exep.tile([128, ROWS_PAD], F32, tag="exe_sb")
        for g in range(NG):
            rsl = slice(g * GR, (g + 1) * GR)
            s1B_p = ps_s1b.tile([128, GR], F32, tag="s1b")
            nc.tensor.matmul(s1B_p[:], lhsT=wa1mat[:], rhs=exT_sb[:, rsl],
                             start=True, stop=True)
            nc.vector.tensor_copy(s1B_sb[:, rsl], s1B_p[:])
            exe_p = ps_exe.tile([128, GR], F32, tag="exe")
            nc.tensor.matmul(exe_p[:], lhsT=emb_sb[:], rhs=exT_sb[:, rsl],
                             start=True, stop=True)
            nc.vector.tensor_copy(exe_sb[:, rsl], exe_p[:])

        # per-chunk full-width: e = prelu(s1B + s2_c); t = exp(e); tm = t*adjT_c
        tmT_sb = tmtp.tile([128, NCH, ROWS_PAD], BF16, tag="tmT")
        for c in range(NCH):
            el_sb = elp.tile([128, ROWS_PAD], F32, tag="el")
            if sim_safe:
                sa = elp.tile([128, 1], F32, tag="sa")
                nc.vector.tensor_scalar_mul(sa[:], s2_sb[:, c:c + 1], LEAKY_SLOPE)
                sb_ = elp.tile([128, 1], F32, tag="sb")
                nc.vector.tensor_scalar_mul(sb_[:], s2_sb[:, c:c + 1],
                                            1.0 - LEAKY_SLOPE)
                e02 = elp.tile([128, ROWS_PAD], F32, tag="el")
                nc.scalar.activation(e02[:], s1B_sb[:], AF.Identity,
                                     bias=sa[:], scale=LEAKY_SLOPE)
                r08 = elp.tile([128, ROWS_PAD], F32, tag="el")
                nc.scalar.activation(r08[:], s1B_sb[:], AF.Relu,
                                     bias=sb_[:], scale=1.0 - LEAKY_SLOPE)
                nc.vector.tensor_add(el_sb[:], e02[:], r08[:])
            else:
                nc.scalar.activation(el_sb[:], s1B_sb[:], AF.Prelu,
                                     bias=s2_sb[:, c:c + 1], scale=1.0,
                                     alpha=LEAKY_SLOPE)
            t_bf = tp_.tile([128, ROWS_PAD], BF16, tag="texp")
            nc.scalar.activation(t_bf[:], el_sb[:], AF.Exp)
            nc.vector.tensor_tensor(tmT_sb[:, c, :], t_bf[:], adjT_sb[:, c, :],
                                    ALU.mult)

        # rowsumB [p, r] = ONES.T @ tm (all partitions identical), per group;
        # copied into a full-width strip, then recipB = exp(-ln(.)) full-width.
        rsb_sb = rcb.tile([128, ROWS_PAD], F32, tag="strip")
        for g in range(NG):
            rsl = slice(g * GR, (g + 1) * GR)
            rsb_p = ps_rsb.tile([128, GR], F32, tag="rsb")
            for c in range(NCH):
                nc.tensor.matmul(rsb_p[:], lhsT=ones_sb[:], rhs=tmT_sb[:, c, rsl],
                                 start=(c == 0), stop=(c == NCH - 1))
            nc.vector.tensor_copy(rsb_sb[:, rsl], rsb_p[:])
        lnb = rcb.tile([128, ROWS_PAD], F32, tag="strip")
        nc.scalar.activation(lnb[:], rsb_sb[:], AF.Ln)
        rcp = rcb.tile([128, ROWS_PAD], F32, tag="strip")
        nc.scalar.activation(rcp[:], lnb[:], AF.Exp, scale=-1.0)

        # aggT [fo, r] = sum_c kcWh_c.T @ tm_c, per group -> m1 = agg*exEh
        m1 = fin.tile([128, ROWS_PAD], F32, tag="fwork")
        for g in range(NG):
            rsl = slice(g * GR, (g + 1) * GR)
            agg_p = ps_agg.tile([128, GR], F32, tag="agg")
            for c in range(NCH):
                nc.tensor.matmul(agg_p[:],
                                 lhsT=kcWh_bf[:, c * 128:(c + 1) * 128],
                                 rhs=tmT_sb[:, c, rsl],
                                 start=(c == 0), stop=(c == NCH - 1))
            nc.vector.tensor_tensor(m1[:, rsl], agg_p[:], exe_sb[:, rsl], ALU.mult)

        # x = m1 * recipB ; out = elu(x) = max(x,0) + exp(min(x,0)) - 1
        x_sb = fin.tile([128, ROWS_PAD], F32, tag="fwork")
        nc.vector.tensor_tensor(x_sb[:], m1[:], rcp[:], ALU.mult)
        xm = fin.tile([128, ROWS_PAD], F32, tag="fwork")
        nc.gpsimd.tensor_scalar_min(xm[:], x_sb[:], 0.0)
        en = fin.tile([128, ROWS_PAD], F32, tag="fwork")
        nc.scalar.activation(en[:], xm[:], AF.Exp)
        xp = fin.tile([128, ROWS_PAD], F32, tag="fwork")
        nc.gpsimd.tensor_scalar_max(xp[:], x_sb[:], 0.0)
        o_sb = fin.tile([128, ROWS_PAD], F32, tag="fwork")
        nc.vector.scalar_tensor_tensor(
            out=o_sb[:], in0=en[:], scalar=1.0, in1=xp[:],
            op0=ALU.subtract, op1=ALU.add)
        nc.sync.dma_start(out[:], o_sb[:])


def build_nc(sim_safe: bool = False, reps: int = 1):
    nc = bacc.Bacc("TRN2", target_bir_lowering=False, debug=False,
                   enable_asserts=False)
    ins = {
        "adjT": nc.dram_tensor("adjT", [N_KC, ROWS_PAD], BF16,
                               kind="ExternalInput").ap(),
        "exT": nc.dram_tensor("exT", [F, ROWS_PAD], BF16,
                              kind="ExternalInput").ap(),
        "kc": nc.dram_tensor("kc", [N_KC, F], F32, kind="ExternalInput").ap(),
        "w1": nc.dram_tensor("w1", [F, F], F32, kind="ExternalInput").ap(),
        "emb": nc.dram_tensor("emb", [F, F], BF16, kind="ExternalInput").ap(),
        "a1": nc.dram_tensor("a1", [F, 1], F32, kind="ExternalInput").ap(),
        "a2": nc.dram_tensor("a2", [F, 1], F32, kind="ExternalInput").ap(),
        "idf": nc.dram_tensor("idf", [128, 128], F32, kind="ExternalInput").ap(),
        "onesb": nc.dram_tensor("onesb", [128, 128], BF16,
                                kind="ExternalInput").ap(),
    }
    outs = {
        "outT": nc.dram_tensor("outT", [F, ROWS_PAD], F32,
                               kind="ExternalOutput").ap(),
    }
    with ExitStack() as ctx:
        tc = ctx.enter_context(tile.TileContext(nc))
        build_kernel_body(ctx, tc, ins, outs, sim_safe=sim_safe, reps=reps)
    nc.finalize()
    return nc


def make_in_maps(exercise_h, kc_h, adj_exercise_kc, W1, E, a):
    bf = ml_dtypes.bfloat16
    common = {
        "kc": np.ascontiguousarray(kc_h, dtype=np.float32),
        "w1": np.ascontiguousarray(W1, dtype=np.float32),
        "emb": np.ascontiguousarray(E.astype(bf)),
        "a1": np.ascontiguousarray(a[:F].reshape(F, 1), dtype=np.float32),
        "a2": np.ascontiguousarray(a[F:].reshape(F, 1), dtype=np.float32),
        "idf": np.eye(128, dtype=np.float32),
        "onesb": np.ones((128, 128), dtype=bf),
    }
    in_maps = []
    for i in range(N_CORES):
        sl = slice(i * ROWS, (i + 1) * ROWS)
        adjT = np.ones((N_KC, ROWS_PAD), dtype=bf)
        adjT[:, :ROWS] = (adj_exercise_kc[sl] > 0).astype(bf).T
        exT = np.zeros((F, ROWS_PAD), dtype=bf)
        exT[:, :ROWS] = exercise_h[sl].astype(bf).T
        in_maps.append({"adjT": np.ascontiguousarray(adjT),
                        "exT": np.ascontiguousarray(exT), **common})
    return in_maps


_NC_CACHE = []


def kernel(exercise_h, kc_h, adj_exercise_kc, W1, E, a, **run_kwargs):
    exercise_h = np.asarray(exercise_h)
    kc_h = np.asarray(kc_h)
    adj_exercise_kc = np.asarray(adj_exercise_kc)
    W1 = np.asarray(W1)
    E = np.asarray(E)
    a = np.asarray(a)
    if not _NC_CACHE:
        _NC_CACHE.append(build_nc())
    nc = _NC_CACHE[0]
    in_maps = make_in_maps(exercise_h, kc_h, adj_exercise_kc, W1, E, a)
    res = run_bass_kernel_spmd(nc, in_maps, core_ids=list(range(N_CORES)),
                               **run_kwargs)
    out = np.concatenate(
        [res.results[i]["outT"][:, :ROWS].T for i in range(N_CORES)], axis=0)
    if run_kwargs:
        kernel.last_results = res
    return np.ascontiguousarray(out, dtype=np.float32)


# revision 37
# speedup vs baseline: 1.2560x; 1.2560x over previous
"""GAT exercise->KC attention layer on 8 TRN2 NeuronCores.

Sharding: exercise rows split 8 ways (2500/core, padded to 2560); kc_h/W1/E/a
replicated. Row-local softmax + aggregation => no collectives.

Layout: everything on-chip runs "transposed" — kc chunks (8 x 128) on
partitions, exercise rows on the free axis. The host ships adj and ex already
transposed, so the kernel needs no per-tile PE transposes at all:

  s1B  [p, r] = W_A1MAT.T @ exT          (every partition = s1[r])
  e_c  [k_c, r] = prelu(s1B + s2[k_c])   (ACT, per-partition bias)
  t_c  = exp(e_c) -> bf16                (ACT)
  tm_c = t_c * adjT_c                    (DVE, bf16)
  rowsumB [p, r] = ONES.T @ tm           (every partition = rowsum[r])
  recipB = exp(-ln(rowsumB))             (ACT; vector.reciprocal is per-lane)
  aggT [fo, r] = sum_c kcWh_c.T @ tm_c   (bf16 matmuls)
  out  = elu(aggT * exEhT * recipB)      ([fo, r]; host transposes back)
"""

import numpy as np
import ml_dtypes
from contextlib import ExitStack

from concourse import bass, bacc, tile, mybir
from concourse.bass_utils import run_bass_kernel_spmd

F32 = mybir.dt.float32
BF16 = mybir.dt.bfloat16

N_EX, N_KC, F = 20000, 1024, 128
N_CORES = 8
ROWS = N_EX // N_CORES          # 2500 rows per core
ROWS_PAD = 2560                 # pad -> uniform tiles
GR = 512                        # rows per group (1 PSUM bank for f32 outputs)
NG = (ROWS_PAD + GR - 1) // GR  # 5 groups
NCH = N_KC // 128               # 8 kc chunks

LEAKY_SLOPE = 0.2
AF = mybir.ActivationFunctionType
ALU = mybir.AluOpType


def build_kernel_body(ctx: ExitStack, tc: "tile.TileContext", ins: dict, outs: dict,
                      sim_safe: bool = False, reps: int = 1):
    nc = tc.nc
    adjT = ins["adjT"]      # [N_KC, ROWS_PAD] bf16 (0/1; pad rows = 1)
    exT = ins["exT"]        # [F, ROWS_PAD] bf16 (pad rows = 0)
    kc = ins["kc"]          # [N_KC, F] f32
    w1 = ins["w1"]          # [F, F] f32
    emb = ins["emb"]        # [F, F] bf16  (E matrix, pre-cast)
    a1 = ins["a1"]          # [F, 1] f32
    a2 = ins["a2"]          # [F, 1] f32
    idf = ins["idf"]        # [128, 128] f32 identity
    onesb = ins["onesb"]    # [128, 128] bf16 all-ones
    out = outs["outT"]      # [F, ROWS_PAD] f32 (host transposes back)

    const = ctx.enter_context(tc.tile_pool(name="const", bufs=1))
    pctx = ctx.enter_context(ExitStack())
    prolog = pctx.enter_context(tc.tile_pool(name="prolog", bufs=2))
    pp = pctx.enter_context(tc.tile_pool(name="pp", bufs=2, space="PSUM"))

    # ---- constants ----
    w1_sb = const.tile([F, F], F32, tag="w1")
    nc.sync.dma_start(w1_sb[:], w1)
    emb_sb = const.tile([F, F], BF16, tag="emb")
    nc.sync.dma_start(emb_sb[:], emb)
    a1_sb = const.tile([F, 1], F32, tag="a1")
    nc.sync.dma_start(a1_sb[:], a1)
    a2_sb = const.tile([F, 1], F32, tag="a2")
    nc.sync.dma_start(a2_sb[:], a2)
    idf_sb = const.tile([128, 128], F32, tag="idf")
    nc.sync.dma_start(idf_sb[:], idf)
    ones_sb = const.tile([128, 128], BF16, tag="onesb")
    nc.sync.dma_start(ones_sb[:], onesb)

    # ---- kc-derived constants ----
    # kc DMA comes BEFORE the bulk adjT/exT loads: the serial prologue chain
    # (kc_hT -> kcWh/kcWhT -> s2/wa1mat) gates the first Prelu.
    kc_all = prolog.tile([128, NCH, F], F32, tag="kc_all")
    nc.sync.dma_start(kc_all[:], kc.rearrange("(c p) f -> p c f", p=128))

    # exercise/adj shards (already transposed on host)
    exT_sb = const.tile([F, ROWS_PAD], BF16, tag="exT")
    nc.sync.dma_start(exT_sb[:], exT)
    adjT_sb = const.tile([128, NCH, ROWS_PAD], BF16, tag="adjT")
    adjT_r = adjT.rearrange("(c p) r -> p c r", p=128)
    for c in range(0, NCH, 2):
        nc.sync.dma_start(adjT_sb[:, c:c + 2, :], adjT_r[:, c:c + 2, :])
    kc_hT = prolog.tile([F, N_KC], F32, tag="kc_hT")
    for c in range(NCH):
        tp = pp.tile([128, 128], F32, tag="ptrans")
        nc.tensor.transpose(tp[:], kc_all[:, c, :], idf_sb[:])
        nc.vector.tensor_copy(kc_hT[:, c * 128:(c + 1) * 128], tp[:])

    # kcWh chunks [k_c, fo] bf16 (aggregation lhsT)
    kcWh_bf = const.tile([128, N_KC], BF16, tag="kcWh")
    for c in range(NCH):
        mp = pp.tile([128, 128], F32, tag="ptrans")
        nc.tensor.matmul(mp[:], lhsT=kc_hT[:, c * 128:(c + 1) * 128], rhs=w1_sb[:],
                         start=True, stop=True)
        nc.scalar.copy(kcWh_bf[:, c * 128:(c + 1) * 128], mp[:])

    # kcWhT [fo, k] f32 -> s2 chunks [k_c, 1] f32 (prelu bias)
    kcWhT = prolog.tile([F, N_KC], F32, tag="kcWhT")
    for h in range(2):
        sl = slice(h * 512, (h + 1) * 512)
        mp = pp.tile([128, 512], F32, tag="pwide")
        nc.tensor.matmul(mp[:], lhsT=w1_sb[:], rhs=kc_hT[:, sl], start=True, stop=True)
        nc.vector.tensor_copy(kcWhT[:, sl], mp[:])
    s2_sb = const.tile([128, NCH], F32, tag="s2")
    for c in range(NCH):
        sp = pp.tile([128, 128], F32, tag="ptrans")
        nc.tensor.matmul(sp[:, :1], lhsT=kcWhT[:, c * 128:(c + 1) * 128],
                         rhs=a2_sb[:], start=True, stop=True)
        nc.vector.tensor_copy(s2_sb[:, c:c + 1], sp[:, :1])

    # w_a1 = W1 @ a1 replicated into a [fi, 128] bf16 stationary: one matmul
    # then broadcasts s1 across partitions for free.
    w1t_p = pp.tile([128, 128], F32, tag="ptrans")
    nc.tensor.transpose(w1t_p[:], w1_sb[:], idf_sb[:])
    w1t_sb = prolog.tile([F, F], F32, tag="w1t")
    nc.vector.tensor_copy(w1t_sb[:], w1t_p[:])
    wa1_p = pp.tile([128, 128], F32, tag="ptrans")
    nc.tensor.matmul(wa1_p[:, :1], lhsT=w1t_sb[:], rhs=a1_sb[:],
                     start=True, stop=True)
    wa1_col = prolog.tile([F, 1], F32, tag="wa1_col")
    nc.vector.tensor_copy(wa1_col[:], wa1_p[:, :1])
    onesf = prolog.tile([128, 128], F32, tag="onesf")
    nc.vector.memset(onesf[:], 1.0)
    wa1mat = const.tile([F, F], BF16, tag="wa1mat")
    nc.scalar.activation(wa1mat[:], onesf[:], AF.Copy, scale=wa1_col[:])

    pctx.close()

    # ---- main-loop pools (SBUF is tight: big [128, 2560] strips tag-share) ----
    elp = ctx.enter_context(tc.tile_pool(name="elp", bufs=3 if sim_safe else 1))
    tp_ = ctx.enter_context(tc.tile_pool(name="tp", bufs=2))
    tmtp = ctx.enter_context(tc.tile_pool(name="tmtp", bufs=1))
    fin = ctx.enter_context(tc.tile_pool(name="fin", bufs=4))
    exep = ctx.enter_context(tc.tile_pool(name="exep", bufs=1))
    rcb = ctx.enter_context(tc.tile_pool(name="rcb", bufs=2))
    ps_s1b = ctx.enter_context(tc.tile_pool(name="ps_s1b", bufs=2, space="PSUM"))
    ps_rsb = ctx.enter_context(tc.tile_pool(name="ps_rsb", bufs=2, space="PSUM"))
    ps_agg = ctx.enter_context(tc.tile_pool(name="ps_agg", bufs=2, space="PSUM"))
    ps_exe = ctx.enter_context(tc.tile_pool(name="ps_exe", bufs=2, space="PSUM"))

    for it in range(reps):
        # s1B [p, r] / exEhT [fo, r]: per-group matmuls (PSUM bank = 512 f32),
        # each copied into a full-width SBUF strip right away.
        s1B_sb = rcb.tile([128, ROWS_PAD], F32, tag="strip")
        for g in range(NG):
            rsl = slice(g * GR, (g + 1) * GR)
            s1B_p = ps_s1b.tile([128, GR], F32, tag="s1b")
            nc.tensor.matmul(s1B_p[:], lhsT=wa1mat[:], rhs=exT_sb[:, rsl],
                             start=True, stop=True)
            nc.vector.tensor_copy(s1B_sb[:, rsl], s1B_p[:])

        # per-chunk full-width: e = prelu(s1B + s2_c); t = exp(e); tm = t*adjT_c
        tmT_sb = tmtp.tile([128, NCH, ROWS_PAD], BF16, tag="tmT")
        for c in range(NCH):
            el_sb = elp.tile([128, ROWS_PAD], F32, tag="el")
            if sim_safe:
                sa = elp.tile([128, 1], F32, tag="sa")
                nc.vector.tensor_scalar_mul(sa[:], s2_sb[:, c:c + 1], LEAKY_SLOPE)
                sb_ = elp.tile([128, 1], F32, tag="sb")
                nc.vector.tensor_scalar_mul(sb_[:], s2_sb[:, c:c + 1],
                                            1.0 - LEAKY_SLOPE)
                e02 = elp.tile([128, ROWS_PAD], F32, tag="el")
                nc.scalar.activation(e02[:], s1B_sb[:], AF.Identity,
                                     bias=sa[:], scale=LEAKY_SLOPE)
                r08 = elp.tile([128, ROWS_PAD], F32, tag="el")
                nc.scalar.activation(r08[:], s1B_sb[:], AF.Relu,
                                     bias=sb_[:], scale=1.0 - LEAKY_SLOPE)
                nc.vector.tensor_add(el_sb[:], e02[:], r08[:])
            else:
                nc.scalar.activation(el_sb[:], s1B_sb[:], AF.Prelu,
                                     bias=s2_sb[:, c:c + 1], scale=1.0,
                                     alpha=LEAKY_SLOPE)
            t_bf = tp_.tile([128, ROWS_PAD], BF16, tag="texp")
            nc.scalar.activation(t_bf[:], el_sb[:], AF.Exp)
            nc.vector.tensor_tensor(tmT_sb[:, c, :], t_bf[:], adjT_sb[:, c, :],
                                    ALU.mult)

        # exEhT (needed only by m1, off the critical head)
        exe_sb = exep.tile([128, ROWS_PAD], F32, tag="exe_sb")
        for g in range(NG):
            rsl = slice(g * GR, (g + 1) * GR)
            exe_p = ps_exe.tile([128, GR], F32, tag="exe")
            nc.tensor.matmul(exe_p[:], lhsT=emb_sb[:], rhs=exT_sb[:, rsl],
                             start=True, stop=True)
            nc.vector.tensor_copy(exe_sb[:, rsl], exe_p[:])

        # rowsumB [p, r] = ONES.T @ tm (all partitions identical), per group;
        # copied into a full-width strip, then recipB = exp(-ln(.)) full-width.
        rsb_sb = rcb.tile([128, ROWS_PAD], F32, tag="strip")
        for g in range(NG):
            rsl = slice(g * GR, (g + 1) * GR)
            rsb_p = ps_rsb.tile([128, GR], F32, tag="rsb")
            for c in range(NCH):
                nc.tensor.matmul(rsb_p[:], lhsT=ones_sb[:], rhs=tmT_sb[:, c, rsl],
                                 start=(c == 0), stop=(c == NCH - 1))
            nc.vector.tensor_copy(rsb_sb[:, rsl], rsb_p[:])
        lnb = rcb.tile([128, ROWS_PAD], F32, tag="strip")
        nc.scalar.activation(lnb[:], rsb_sb[:], AF.Ln)
        rcp = rcb.tile([128, ROWS_PAD], F32, tag="strip")
        nc.scalar.activation(rcp[:], lnb[:], AF.Exp, scale=-1.0)

        # aggT [fo, r] = sum_c kcWh_c.T @ tm_c, per group -> m1 = agg*exEh
        m1 = fin.tile([128, ROWS_PAD], F32, tag="fwork")
        for g in range(NG):
            rsl = slice(g * GR, (g + 1) * GR)
            agg_p = ps_agg.tile([128, GR], F32, tag="agg")
            for c in range(NCH):
                nc.tensor.matmul(agg_p[:],
                                 lhsT=kcWh_bf[:, c * 128:(c + 1) * 128],
                                 rhs=tmT_sb[:, c, rsl],
                                 start=(c == 0), stop=(c == NCH - 1))
            nc.vector.tensor_tensor(m1[:, rsl], agg_p[:], exe_sb[:, rsl], ALU.mult)

        # x = m1 * recipB ; out = elu(x) = max(x,0) + exp(min(x,0)) - 1
        x_sb = fin.tile([128, ROWS_PAD], F32, tag="fwork")
        nc.vector.tensor_tensor(x_sb[:], m1[:], rcp[:], ALU.mult)
        xm = fin.tile([128, ROWS_PAD], F32, tag="fwork")
        nc.vector.tensor_scalar_min(xm[:], x_sb[:], 0.0)
        en = fin.tile([128, ROWS_PAD], F32, tag="fwork")
        nc.scalar.activation(en[:], xm[:], AF.Exp)
        xp = fin.tile([128, ROWS_PAD], F32, tag="fwork")
        nc.gpsimd.tensor_scalar_max(xp[:], x_sb[:], 0.0)
        o_sb = fin.tile([128, ROWS_PAD], F32, tag="fwork")
        nc.vector.scalar_tensor_tensor(
            out=o_sb[:], in0=en[:], scalar=1.0, in1=xp[:],
            op0=ALU.subtract, op1=ALU.add)
        nc.sync.dma_start(out[:], o_sb[:])


def build_nc(sim_safe: bool = False, reps: int = 1):
    nc = bacc.Bacc("TRN2", target_bir_lowering=False, debug=False,
                   enable_asserts=False)
    ins = {
        "adjT": nc.dram_tensor("adjT", [N_KC, ROWS_PAD], BF16,
                               kind="ExternalInput").ap(),
        "exT": nc.dram_tensor("exT", [F, ROWS_PAD], BF16,
                              kind="ExternalInput").ap(),
        "kc": nc.dram_tensor("kc", [N_KC, F], F32, kind="ExternalInput").ap(),
        "w1": nc.dram_tensor("w1", [F, F], F32, kind="ExternalInput").ap(),
        "emb": nc.dram_tensor("emb", [F, F], BF16, kind="ExternalInput").ap(),
        "a1": nc.dram_tensor("a1", [F, 1], F32, kind="ExternalInput").ap(),
        "a2": nc.dram_tensor("a2", [F, 1], F32, kind="ExternalInput").ap(),
        "idf": nc.dram_tensor("idf", [128, 128], F32, kind="ExternalInput").ap(),
        "onesb": nc.dram_tensor("onesb", [128, 128], BF16,
                                kind="ExternalInput").ap(),
    }
    outs = {
        "outT": nc.dram_tensor("outT", [F, ROWS_PAD], F32,
                               kind="ExternalOutput").ap(),
    }
    with ExitStack() as ctx:
        tc = ctx.enter_context(tile.TileContext(nc))
        build_kernel_body(ctx, tc, ins, outs, sim_safe=sim_safe, reps=reps)
    nc.finalize()
    return nc


def make_in_maps(exercise_h, kc_h, adj_exercise_kc, W1, E, a):
    bf = ml_dtypes.bfloat16
    common = {
        "kc": np.ascontiguousarray(kc_h, dtype=np.float32),
        "w1": np.ascontiguousarray(W1, dtype=np.float32),
        "emb": np.ascontiguousarray(E.astype(bf)),
        "a1": np.ascontiguousarray(a[:F].reshape(F, 1), dtype=np.float32),
        "a2": np.ascontiguousarray(a[F:].reshape(F, 1), dtype=np.float32),
        "idf": np.eye(128, dtype=np.float32),
        "onesb": np.ones((128, 128), dtype=bf),
    }
    in_maps = []
    for i in range(N_CORES):
        sl = slice(i * ROWS, (i + 1) * ROWS)
        adjT = np.ones((N_KC, ROWS_PAD), dtype=bf)
        adjT[:, :ROWS] = (adj_exercise_kc[sl] > 0).astype(bf).T
        exT = np.zeros((F, ROWS_PAD), dtype=bf)
        exT[:, :ROWS] = exercise_h[sl].astype(bf).T
        in_maps.append({"adjT": np.ascontiguousarray(adjT),
                        "exT": np.ascontiguousarray(exT), **common})
    return in_maps


_NC_CACHE = []


def kernel(exercise_h, kc_h, adj_exercise_kc, W1, E, a, **run_kwargs):
    exercise_h = np.asarray(exercise_h)
    kc_h = np.asarray(kc_h)
    adj_exercise_kc = np.asarray(adj_exercise_kc)
    W1 = np.asarray(W1)
    E = np.asarray(E)
    a = np.asarray(a)
    if not _NC_CACHE:
        _NC_CACHE.append(build_nc())
    nc = _NC_CACHE[0]
    in_maps = make_in_maps(exercise_h, kc_h, adj_exercise_kc, W1, E, a)
    res = run_bass_kernel_spmd(nc, in_maps, core_ids=list(range(N_CORES)),
                               **run_kwargs)
    out = np.concatenate(
        [res.results[i]["outT"][:, :ROWS].T for i in range(N_CORES)], axis=0)
    if run_kwargs:
        kernel.last_results = res
    return np.ascontiguousarray(out, dtype=np.float32)
